# revision 10
# baseline (speedup 1.0000x reference)
"""GPT2-style decision-transformer forward pass on 8 TRN2 NeuronCores.

Data-parallel: 16 sequences -> 2 per core. Each core runs the full
4-layer transformer on its 2 sequences (602 tokens) and reduces its
loss-sum / correct-count to a [1,2] output; the host sums the 8 partials.

All large matmuls (qkv, v, fc, mproj, aproj) run in fp8e4 with
DoubleRow perf mode (two 128-deep k-tiles per matmul, 2x PE throughput).
Weights are host-scaled by WS=16 (keeps small entries out of the fp8
subnormal range) and the 1/WS is folded into the psum->sbuf copy.
Activations feeding fp8 matmuls (xnT, attnT, geluT) are stored as fp8
"pair" tiles [128, 2, T] where dim1 indexes the two k-tiles of a
DoubleRow pair.  The token axis is padded to 608 (seq stride 304) so
fp8 DoubleRow ldweights access patterns stay 4-byte aligned.

Attention uses a transposed-scores formulation in bf16: scoresT = K^T Q
per k-chunk (kc1/kc2 share one psum bank so exp is 2 ops per head),
then a DIRECT flipped PV: attnT[feat, q] = V^T @ probsT accumulated
straight into one psum bank (head sub0 rows 0:64, sub1 rows 64:128)
with no transpose-back and no per-chunk normalize copies.  Softmax row
sums come from tiny M=1 ones-matmuls into a per-layer static psum bank
(2 rows per head-pair/seq); 1/rowsum is computed as exp(-ln(rs)) on
the scalar engine (same table set as the attention exp), bounced
through a DRAM scratch row to partition-broadcast it to [128, L], and
applied in one vector multiply that also performs the fp8 conversion
into attnT.  The WS scale on V and the AS fp8 up-scale on attnT are
folded into the reciprocal's exp bias for free.

LayerNorm: bn_stats/bn_aggr on DVE, 1/std as exp(-0.5*ln(var+eps))
(scalar engine, exp/ln table set -- no sqrt table switches), and the
normalize itself as a DVE tensor_scalar (per-partition scale+bias).

All problem biases are zero (setup_inputs uses zeros); bias paths are
only emitted when the host detects a nonzero bias (build cached per
flag set).
"""

import numpy as np
import ml_dtypes
from contextlib import ExitStack

import concourse.bass as bass
import concourse.tile as tile
from concourse import bacc, mybir
from concourse.bass_utils import run_bass_kernel_spmd

F32 = mybir.dt.float32
BF16 = mybir.dt.bfloat16
FP8 = mybir.dt.float8e4
AF = mybir.ActivationFunctionType
ALU = mybir.AluOpType
DR = mybir.MatmulPerfMode.DoubleRow

B, CTX, D, H, NL, DFF, G, NA = 16, 100, 1024, 16, 4, 4096, 9, 5
L = 3 * CTX + 1          # 301
HD = D // H              # 64
LN_EPS = 1e-5
LS = 0.1
NCORES = 8
S = B // NCORES          # 2 seqs per core
LP = 304                 # per-seq padded length (4-aligned for fp8 lhsT)
T = S * LP               # 608 padded tokens per core
NTOK = [128, 128, 128, 128, 96]   # token tile sizes (padded axis)
NT = len(NTOK)
EMB = G * G + NA + 1     # 87 combined embedding rows
NEG = -60.0              # additive causal mask value (exp(-60) ~ 9e-27)
WS = 16.0                # fp8 weight pre-scale
IWS = 1.0 / WS
AS = 8.0                 # attnT fp8 up-scale (keeps attn out of subnormals)
RCP_BIAS = float(np.log(AS / WS))   # exp(-ln rs + RCP_BIAS) = AS/(WS*rs)


def _bf16(x):
    return np.asarray(x, dtype=ml_dtypes.bfloat16)


def _f8(x):
    return np.asarray(x, dtype=ml_dtypes.float8_e4m3)


def _f32(x):
    return np.ascontiguousarray(np.asarray(x, dtype=np.float32))


def _pack_pairs(w, nkp, nfg):
    """[K, F] -> [nkp, 128, nfg, 2, 512] DoubleRow pair layout.
    pack[kp, p, fg, j, f] = w[(2*kp + j)*128 + p, fg*512 + f]."""
    K, F = w.shape
    assert K == nkp * 256 and F == nfg * 512
    return np.ascontiguousarray(
        w.reshape(nkp, 2, 128, nfg, 512).transpose(0, 2, 3, 1, 4))


# --------------------------------------------------------------------------
# graph builder
# --------------------------------------------------------------------------

def build(flags=(False, False, False, False, False, False)):
    vbias_f, apbias_f, mpbias_f, predbias_f, qkvb_f, fcb_f = flags
    any_ones = vbias_f or apbias_f or mpbias_f or predbias_f
    nc = bacc.Bacc("TRN2", target_bir_lowering=False, debug=False,
                   enable_asserts=True, num_devices=NCORES)

    def inp(name, shape, dt):
        return nc.dram_tensor(name, list(shape), dt, kind="ExternalInput").ap()

    d_membT = inp("m_embT", (EMB, T), BF16)
    d_temb = inp("t_emb", (EMB, D), BF16)
    d_wpe = inp("wpe", (T, D), F32)
    d_wqkv, d_bqkv, d_wap = [], [], []
    d_wfc, d_bfc, d_wmp = [], [], []
    d_bv, d_bap, d_bmp = [], [], []
    for i in range(NL):
        d_wqkv.append(inp(f"w_qkv_{i}", (4, 128, 6, 2, 512), FP8))
        if qkvb_f:
            d_bqkv.append(inp(f"b_qkv_{i}", (128, 16), F32))
        d_wap.append(inp(f"w_aproj_{i}", (4, 128, 2, 2, 512), FP8))
        d_wfc.append(inp(f"w_fc_{i}", (4, 128, 8, 2, 512), FP8))
        if fcb_f:
            d_bfc.append(inp(f"b_fc_{i}", (128, 32), F32))
        d_wmp.append(inp(f"w_mproj_{i}", (16, 128, 2, 2, 512), FP8))
        if vbias_f:
            d_bv.append(inp(f"b_v_{i}", (1, D), BF16))
        if apbias_f:
            d_bap.append(inp(f"b_aproj_{i}", (1, D), BF16))
        if mpbias_f:
            d_bmp.append(inp(f"b_mproj_{i}", (1, D), BF16))
    d_wpred = inp("w_pred", (128, 8, NA), BF16)   # host pre-laid-out
    if predbias_f:
        d_bpred = inp("b_pred", (1, NA), BF16)
    d_tgt = inp("tgt_oh", (T, NA), F32)
    d_smask = inp("smask", (128, NT), F32)
    d_ident8 = inp("ident_f8", (128, 128), FP8)
    d_ident5 = inp("ident5", (NA, NA), F32)
    d_tri = inp("tri", (128, 128), F32)
    d_maskB = inp("maskB", (128, 224), F32)
    if any_ones:
        d_ones1 = inp("ones1", (1, 512), BF16)
    d_ones128 = inp("ones128", (128, 1), F32)
    d_out = nc.dram_tensor("out", [1, 2], F32, kind="ExternalOutput").ap()

    with tile.TileContext(nc) as tc, ExitStack() as ctx:
        # ---------------- pools
        const_p = ctx.enter_context(tc.tile_pool(name="const", bufs=1))
        pers_p = ctx.enter_context(tc.tile_pool(name="pers", bufs=1))
        w_p = ctx.enter_context(tc.tile_pool(name="w", bufs=36))
        bias_p = ctx.enter_context(tc.tile_pool(name="bias", bufs=2))
        xn_p = ctx.enter_context(tc.tile_pool(name="xn", bufs=3))
        st_p = ctx.enter_context(tc.tile_pool(name="st", bufs=24))
        pr_p = ctx.enter_context(tc.tile_pool(name="pr", bufs=12))
        rsb_p = ctx.enter_context(tc.tile_pool(name="rsb", bufs=3))
        ps_p = ctx.enter_context(tc.tile_pool(name="ps", bufs=8, space="PSUM"))
        dscr_p = ctx.enter_context(tc.tile_pool(name="dscr", bufs=4,
                                                space="DRAM"))

        def psum(pdim=128, fdim=512, dt=F32):
            t = ps_p.tile([128, 512], F32, tag="ps")
            return t[:pdim, :fdim]

        # ---------------- constants
        ident8 = const_p.tile([128, 128], FP8, tag="ident8")
        nc.sync.dma_start(ident8[:], d_ident8[:, :])
        ident5 = const_p.tile([NA, NA], F32, tag="ident5")
        nc.sync.dma_start(ident5[:], d_ident5[:, :])
        tri = const_p.tile([128, 128], F32, tag="tri")
        nc.sync.dma_start(tri[:], d_tri[:, :])
        maskB = const_p.tile([128, 224], F32, tag="maskB")
        nc.sync.dma_start(maskB[:], d_maskB[:, :])
        if any_ones:
            ones1 = const_p.tile([1, 512], BF16, tag="ones1")
            nc.sync.dma_start(ones1[:], d_ones1[:, :])
        ones128 = const_p.tile([128, 1], F32, tag="ones128")
        nc.sync.dma_start(ones128[:], d_ones128[:, :])
        onesb = const_p.tile([128, 1], BF16, tag="onesb")
        nc.vector.memset(onesb[:], 1.0)
        if predbias_f:
            bpred = const_p.tile([1, NA], BF16, tag="bpred")
            nc.sync.dma_start(bpred[:], d_bpred[:, :])
        smask = const_p.tile([128, NT], F32, tag="smask")
        nc.sync.dma_start(smask[:], d_smask[:, :])
        wpred = const_p.tile([128, 8, NA], BF16, tag="wpred")
        nc.sync.dma_start(wpred[:], d_wpred[:, :, :])
        tgt = const_p.tile([128, NT, NA], F32, tag="tgt")
        for tt in range(NT):
            n = NTOK[tt]
            nc.sync.dma_start(tgt[:n, tt, :], d_tgt[tt * 128:tt * 128 + n, :])
        eps_sb = const_p.tile([128, 1], F32, tag="eps")
        nc.vector.memset(eps_sb[:], LN_EPS)
        rcpb_sb = const_p.tile([128, 1], F32, tag="rcpb")
        nc.vector.memset(rcpb_sb[:], RCP_BIAS)
        membT = const_p.tile([EMB, T], BF16, tag="membT")
        nc.sync.dma_start(membT[:], d_membT[:, :])
        temb = const_p.tile([EMB, D], BF16, tag="temb")
        nc.sync.dma_start(temb[:], d_temb[:, :])

        # ---------------- persistent activations
        h = [pers_p.tile([128, D], F32, tag=f"h{i}", name=f"h{i}")
             for i in range(NT)]
        # fp8 pair tiles: dim1 indexes the two k-tiles of a DoubleRow pair
        xnT = [pers_p.tile([128, 2, T], FP8, tag=f"xnT{i}", name=f"xnT{i}")
               for i in range(4)]
        # q/k head-pair tiles: qp[hp][:, 0, :] = q, qp[hp][:, 1, :] = k
        qp = [pers_p.tile([128, 2, T], BF16, tag=f"qp{i}", name=f"qp{i}")
              for i in range(8)]
        vsb = [pers_p.tile([128, 16, 64], BF16, tag=f"vsb{i}", name=f"vsb{i}")
               for i in range(6)]
        attnT = [pers_p.tile([128, 2, T], FP8, tag=f"attnT{i}", name=f"attnT{i}")
                 for i in range(4)]
        geluT = [pers_p.tile([128, 2, T], FP8, tag=f"geluT{i}", name=f"geluT{i}")
                 for i in range(16)]

        # attnT pad columns are never written by attention; zero them once
        for pp in range(4):
            nc.vector.memset(attnT[pp][:, :, L:LP], 0.0)
            nc.vector.memset(attnT[pp][:, :, LP + L:], 0.0)

        # ---------------- embedding: h = wpe_eff + M_embT.T @ T_emb
        for tt in range(NT):
            n = NTOK[tt]
            nc.sync.dma_start(h[tt][:n, :], d_wpe[tt * 128:tt * 128 + n, :])
            for half in range(2):
                ps = psum(n, 512)
                nc.tensor.matmul(ps, membT[:, tt * 128:tt * 128 + n],
                                 temb[:, half * 512:(half + 1) * 512],
                                 start=True, stop=True)
                nc.vector.tensor_tensor(
                    out=h[tt][:n, half * 512:(half + 1) * 512],
                    in0=h[tt][:n, half * 512:(half + 1) * 512],
                    in1=ps, op=ALU.add)

        # ---------------- helpers
        def ln_tile(tt, to_qp=False):
            """LN (pure normalize) on h[tt] -> xn fp8 -> transpose into the
            xnT fp8 pair tiles (or bf16 qp tiles for the final LN)."""
            n = NTOK[tt]
            st6 = st_p.tile([128, 2, 6], F32, tag="st6")
            mv = st_p.tile([128, 2], F32, tag="mv")
            lnv = st_p.tile([128, 1], F32, tag="lnv")
            inv = st_p.tile([128, 1], F32, tag="inv")
            nmi = st_p.tile([128, 1], F32, tag="nmi")
            nc.vector.bn_stats(out=st6[:n, 0, :], in_=h[tt][:n, 0:512])
            nc.vector.bn_stats(out=st6[:n, 1, :], in_=h[tt][:n, 512:1024])
            nc.vector.bn_aggr(out=mv[:n, :], in_=st6[:n, :, :])
            # 1/std = exp(-0.5*ln(var+eps)): stays in the exp/ln table set
            nc.scalar.activation(out=lnv[:n, :], in_=mv[:n, 1:2],
                                 func=AF.Ln, bias=eps_sb[:n, :])
            nc.scalar.activation(out=inv[:n, :], in_=lnv[:n, :],
                                 func=AF.Exp, scale=-0.5)
            nc.vector.scalar_tensor_tensor(
                out=nmi[:n, :], in0=mv[:n, 0:1], scalar=-1.0,
                in1=inv[:n, :], op0=ALU.mult, op1=ALU.mult)
            xn = xn_p.tile([128, D], FP8, tag="xn")
            nc.vector.tensor_scalar(
                out=xn[:n, :], in0=h[tt][:n, :],
                scalar1=inv[:n, 0:1], scalar2=nmi[:n, 0:1],
                op0=ALU.mult, op1=ALU.add)
            c0 = tt * 128
            for bk in range(2):
                # one [128,512] bank <- 4 transposed 128-chunks
                psb = psum(128, 512)
                for q in range(4):
                    dc = 4 * bk + q
                    nc.tensor.matmul(psb[:, q * 128:q * 128 + n],
                                     xn[:n, dc * 128:(dc + 1) * 128],
                                     ident8[:n, :n],
                                     start=(q == 0), stop=(q == 3),
                                     skip_group_check=True)
                for half in range(2):
                    pp = 2 * bk + half
                    src = psb[:, half * 256:(half + 1) * 256].rearrange(
                        "p (j q) -> p j q", j=2)[:, :, :n]
                    dst = (qp[pp] if to_qp else xnT[pp])[:, :, c0:c0 + n]
                    if pp % 2 == 0:
                        nc.vector.tensor_copy(out=dst, in_=src)
                    else:
                        nc.scalar.copy(out=dst, in_=src)

        def qkv_group_weights(li, fg):
            wt = []
            for kp in range(4):
                w = w_p.tile([128, 2, 512], FP8, tag="w")
                nc.sync.dma_start(w[:], d_wqkv[li][kp, :, fg, :, :])
                wt.append(w)
            return wt

        def qkv_pair_mms(wt, fg, bqkv, pr_range):
            """DoubleRow featT matmuls for q/k pair planes of hp = 2*fg+pr."""
            for pr in pr_range:
                hp = 2 * fg + pr
                for j in range(2):
                    fs = 2 * pr + j
                    fq = 2 * hp + j
                    lhs = [wt[kp][:, :, fs * 128:(fs + 1) * 128]
                           for kp in range(4)]
                    ps0 = psum(128, 512)
                    ps1 = psum(128, 96)
                    for kp in range(4):
                        nc.tensor.matmul(ps0, lhs[kp], xnT[kp][:, :, 0:512],
                                         start=(kp == 0), stop=(kp == 3),
                                         perf_mode=DR)
                        nc.tensor.matmul(ps1, lhs[kp], xnT[kp][:, :, 512:608],
                                         start=(kp == 0), stop=(kp == 3),
                                         perf_mode=DR)
                    for ps, sl in ((ps0, slice(0, 512)), (ps1, slice(512, 608))):
                        if qkvb_f:
                            nc.vector.tensor_scalar(
                                out=qp[hp][:, j, sl], in0=ps,
                                scalar1=IWS, scalar2=bqkv[:, fq:fq + 1],
                                op0=ALU.mult, op1=ALU.add)
                        else:
                            nc.vector.tensor_scalar_mul(
                                out=qp[hp][:, j, sl], in0=ps, scalar1=IWS)

        def proj_residual(d_w, inT, nkp, scale, tail=None, bias_sb=None):
            """h += scale * (inT.T @ W) [+ b];  inT fp8 pair tiles."""
            for nh in range(2):
                pss = [psum(NTOK[tt], 512) for tt in range(NT)]
                for blk in range(0, nkp, 8):
                    be = min(blk + 8, nkp)
                    wt = []
                    for kp in range(blk, be):
                        w = w_p.tile([128, 2, 512], FP8, tag="w")
                        nc.sync.dma_start(w[:], d_w[kp, :, nh, :, :])
                        wt.append(w)
                    for tt in range(NT):
                        n = NTOK[tt]
                        for j, kp in enumerate(range(blk, be)):
                            nc.tensor.matmul(
                                pss[tt],
                                inT[kp][:, :, tt * 128:tt * 128 + n],
                                wt[j], start=(kp == 0), stop=(kp == nkp - 1),
                                perf_mode=DR)
                for tt in range(NT):
                    n = NTOK[tt]
                    if bias_sb is not None:
                        # rarely-taken generic path: bias via ones matmul
                        nc.tensor.matmul(pss[tt], ones1[0:1, :n],
                                         bias_sb[0:1, nh * 512:(nh + 1) * 512],
                                         start=False, stop=True,
                                         skip_group_check=True)
                    nc.vector.scalar_tensor_tensor(
                        out=h[tt][:n, nh * 512:(nh + 1) * 512],
                        in0=pss[tt], scalar=scale,
                        in1=h[tt][:n, nh * 512:(nh + 1) * 512],
                        op0=ALU.mult, op1=ALU.add)
                    if nh == 1 and tail is not None:
                        tail(tt)

        # ---------------- transformer layers
        SEQCH = [(0, 128), (128, 128), (256, 45)]   # per-seq k-chunks
        KCW = [128, 128, 45]

        for li in range(NL):
            bqkv = bfc = None
            if qkvb_f:
                bqkv = bias_p.tile([128, 16], F32, tag="bqkv")
                nc.sync.dma_start(bqkv[:], d_bqkv[li][:, :])
            if fcb_f:
                bfc = bias_p.tile([128, 32], F32, tag="bfc")
                nc.sync.dma_start(bfc[:], d_bfc[li][:, :])
            bv = bap = bmp = None
            if vbias_f:
                bv = bias_p.tile([1, D], BF16, tag="bv")
                nc.sync.dma_start(bv[:], d_bv[li][:, :])
            if apbias_f:
                bap = bias_p.tile([1, D], BF16, tag="bap")
                nc.sync.dma_start(bap[:], d_bap[li][:, :])
            if mpbias_f:
                bmp = bias_p.tile([1, D], BF16, tag="bmp")
                nc.sync.dma_start(bmp[:], d_bmp[li][:, :])

            # ---- ln1; v first (swapped DoubleRow matmul producing
            # V[tok, feat] per-seq-chunk), then q,k via featT matmuls
            if li == 0:
                for tt in range(NT):
                    ln_tile(tt)
            for nh in range(2):
                wv = []
                for kp in range(4):
                    w = w_p.tile([128, 2, 512], FP8, tag="w", name="wv")
                    nc.sync.dma_start(w[:], d_wqkv[li][kp, :, 4 + nh, :, :])
                    wv.append(w)
                for s in range(2):
                    for kc in range(3):
                        c0, cw = SEQCH[kc]
                        col = s * LP + c0
                        ps = psum(cw, 512)
                        for kp in range(4):
                            nc.tensor.matmul(ps, xnT[kp][:, :, col:col + cw],
                                             wv[kp], start=(kp == 0),
                                             stop=(kp == 3), perf_mode=DR)
                        if vbias_f:
                            nc.tensor.matmul(ps, ones1[0:1, :cw],
                                             bv[0:1, nh * 512:(nh + 1) * 512],
                                             start=False, stop=True,
                                             skip_group_check=True)
                        # vsb keeps the WS scale; folded into RCP_BIAS
                        nc.vector.tensor_copy(
                            out=vsb[s * 3 + kc][:cw, nh * 8:(nh + 1) * 8, :],
                            in_=ps.rearrange("p (h f) -> p h f", h=8))

            # ---- attention: transposed scores -> exp -> direct flipped PV
            def attn_scores(hp, s):
                """scoresT + exp for both heads of pair hp, seq s.
                kc0 in bank A [128, 301]; kc1+kc2 share bank B
                ([128, 0:173] and [45, 176:221])."""
                off = s * LP
                pb = {}
                for sub in range(2):
                    bp = sub * 64
                    bankA = ps_p.tile([128, 512], F32, tag="ps", name="ps_scA")
                    bankB = ps_p.tile([128, 512], F32, tag="ps", name="ps_scB")
                    place = {0: (bankA, 0, True), 1: (bankB, 0, True),
                             2: (bankB, 176, False)}
                    for kc in range(3):
                        kcw = KCW[kc]
                        qn = L - kc * 128      # q range [kc*128, L)
                        bank, cc0, st = place[kc]
                        nc.tensor.matmul(
                            bank[:kcw, cc0:cc0 + qn],
                            qp[hp][bp:bp + 64, 1,
                                   off + kc * 128: off + kc * 128 + kcw],
                            qp[hp][bp:bp + 64, 0, off + kc * 128: off + L],
                            start=st, stop=True, skip_group_check=True)
                    # causal masks: one TT per bank
                    nc.vector.tensor_tensor(
                        out=bankA[:, 0:128], in0=bankA[:, 0:128],
                        in1=tri[:, :], op=ALU.add)
                    nc.vector.tensor_tensor(
                        out=bankB[:, 0:221], in0=bankB[:, 0:221],
                        in1=maskB[:, 0:221], op=ALU.add)
                    pa = pr_p.tile([128, 304], BF16, tag="probs", name="pbA")
                    nc.scalar.activation(out=pa[:, 0:301], in_=bankA[:, 0:301],
                                         func=AF.Exp)
                    pbt = pr_p.tile([128, 304], BF16, tag="probs", name="pbB")
                    nc.scalar.activation(out=pbt[:, 0:221], in_=bankB[:, 0:221],
                                         func=AF.Exp)
                    pb[sub, 0] = pa
                    pb[sub, 1] = pbt
                return pb

            def attn_pv(hp, s, pb):
                """direct PV into one bank (sub0 rows 0:64, sub1 64:128);
                rowsums via ones-matmuls (sub0 row 0, sub1 row 32 -- matmul
                out base partitions must be 32-aligned); 1/rs = exp(-ln rs);
                DRAM-bounced partition broadcast."""
                off = s * LP
                av = ps_p.tile([128, 512], F32, tag="ps", name="ps_av")
                rsp = ps_p.tile([128, 512], F32, tag="ps", name="ps_rowsum")
                for sub in range(2):
                    bp = sub * 64
                    sl = [(pb[sub, 0], 128, 0, 301, 0),
                          (pb[sub, 1], 128, 0, 173, 128),
                          (pb[sub, 1], 45, 176, 45, 256)]
                    for kc in range(3):
                        t, kcw, cc0, qn, q0 = sl[kc]
                        nc.tensor.matmul(
                            av[bp:bp + 64, q0:q0 + qn],
                            vsb[s * 3 + kc][:kcw, 2 * hp + sub, :],
                            t[:kcw, cc0:cc0 + qn],
                            start=(kc == 0), stop=(kc == 2),
                            skip_group_check=True)
                    for kc in range(3):
                        t, kcw, cc0, qn, q0 = sl[kc]
                        nc.tensor.matmul(
                            rsp[32 * sub:32 * sub + 1, q0:q0 + qn],
                            onesb[:kcw, :], t[:kcw, cc0:cc0 + qn],
                            start=(kc == 0), stop=(kc == 2),
                            skip_group_check=True)
                # one wide ln/exp over rows 0:33 (rows 1:32 are garbage and
                # cost nothing -- ACT time depends on the free dim only)
                lnr = st_p.tile([33, 304], F32, tag="lnr", bufs=3)
                nc.scalar.activation(out=lnr[:, 0:301],
                                     in_=rsp[0:33, 0:301], func=AF.Ln)
                rcp = st_p.tile([33, 304], F32, tag="rcp", bufs=3)
                nc.scalar.activation(out=rcp[:, 0:301], in_=lnr[:, 0:301],
                                     func=AF.Exp, scale=-1.0,
                                     bias=rcpb_sb[:33, :])
                dsc = dscr_p.tile([2, 304], F32, tag="dscr")
                nc.sync.dma_start(dsc[0:1, 0:301], rcp[0:1, 0:301])
                nc.sync.dma_start(dsc[1:2, 0:301], rcp[32:33, 0:301])
                rsb = rsb_p.tile([128, 304], F32, tag="rsb")
                nc.sync.dma_start(rsb[0:64, 0:301],
                                  dsc[0:1, 0:301].to_broadcast((64, 301)))
                nc.sync.dma_start(rsb[64:128, 0:301],
                                  dsc[1:2, 0:301].to_broadcast((64, 301)))
                return av, rsb, off

            def attn_norm(hp, av, rsb, off):
                nc.vector.tensor_tensor(
                    out=attnT[hp // 2][:, hp % 2, off:off + L],
                    in0=av[:, 0:L], in1=rsb[:, 0:L], op=ALU.mult)

            wt = qkv_group_weights(li, 0)
            qkv_pair_mms(wt, 0, bqkv, (0, 1))
            pend_pv = None      # ((hp, s, pb), (hp, s, pb)) awaiting PV
            pend_nm = None      # ((hp, av, rsb, off) x2) awaiting normalize
            for fg in range(4):
                for j, hp in enumerate((2 * fg, 2 * fg + 1)):
                    st0 = attn_scores(hp, 0)
                    st1 = attn_scores(hp, 1)
                    # interleave half of next qkv group's matmuls
                    if fg < 3:
                        if j == 0:
                            wt = qkv_group_weights(li, fg + 1)
                            qkv_pair_mms(wt, fg + 1, bqkv, (0,))
                        else:
                            qkv_pair_mms(wt, fg + 1, bqkv, (1,))
                    nm = None
                    if pend_pv is not None:
                        (p0, p1) = pend_pv
                        nm = ((p0[0],) + attn_pv(*p0),
                              (p1[0],) + attn_pv(*p1))
                    if pend_nm is not None:
                        attn_norm(*pend_nm[0])
                        attn_norm(*pend_nm[1])
                    pend_nm = nm
                    pend_pv = ((hp, 0, st0), (hp, 1, st1))
            (p0, p1) = pend_pv
            nm = ((p0[0],) + attn_pv(*p0), (p1[0],) + attn_pv(*p1))
            if pend_nm is not None:
                attn_norm(*pend_nm[0])
                attn_norm(*pend_nm[1])
            attn_norm(*nm[0])
            attn_norm(*nm[1])

            # ---- attn proj + residual (ln2 fused into the tail)
            proj_residual(d_wap[li], attnT, 4, IWS / AS, tail=ln_tile,
                          bias_sb=bap if apbias_f else None)

            # ---- mlp (next-layer ln1 / lnf fused into mproj's tail)
            for fg in range(8):
                wt = []
                for kp in range(4):
                    w = w_p.tile([128, 2, 512], FP8, tag="w")
                    nc.sync.dma_start(w[:], d_wfc[li][kp, :, fg, :, :])
                    wt.append(w)
                for fs in range(4):
                    fq = fg * 4 + fs
                    gq, gj = fq // 2, fq % 2
                    lhs = [wt[kp][:, :, fs * 128:(fs + 1) * 128]
                           for kp in range(4)]
                    ps0 = psum(128, 512)
                    ps1 = psum(128, 96)
                    for kp in range(4):
                        nc.tensor.matmul(ps0, lhs[kp], xnT[kp][:, :, 0:512],
                                         start=(kp == 0), stop=(kp == 3),
                                         perf_mode=DR)
                        nc.tensor.matmul(ps1, lhs[kp], xnT[kp][:, :, 512:608],
                                         start=(kp == 0), stop=(kp == 3),
                                         perf_mode=DR)
                    for ps, sl in ((ps0, slice(0, 512)), (ps1, slice(512, 608))):
                        if fcb_f:
                            nc.scalar.activation(
                                out=geluT[gq][:, gj, sl], in_=ps,
                                func=AF.Gelu_apprx_tanh,
                                bias=bfc[:, fq:fq + 1], scale=IWS)
                        else:
                            nc.scalar.activation(
                                out=geluT[gq][:, gj, sl], in_=ps,
                                func=AF.Gelu_apprx_tanh, scale=IWS)
            last = (li == NL - 1)
            proj_residual(d_wmp[li], geluT, 16, IWS,
                          tail=(lambda tt: ln_tile(tt, to_qp=True))
                          if last else ln_tile,
                          bias_sb=bmp if mpbias_f else None)

        # ---------------- logits + loss (lnf output sits in qp bf16)
        lg = const_p.tile([NA, T], F32, tag="logits")
        for tch, (t0, tn) in enumerate(((0, 512), (512, 96))):
            ps = psum(NA, tn)
            for dc in range(8):
                nc.tensor.matmul(ps, wpred[:, dc, :],
                                 qp[dc // 2][:, dc % 2, t0:t0 + tn],
                                 start=(dc == 0), stop=(dc == 7))
            if predbias_f:
                nc.tensor.matmul(ps, bpred[0:1, :], ones1[0:1, :tn],
                                 start=False, stop=True,
                                 skip_group_check=True)
            nc.scalar.copy(out=lg[:, t0:t0 + tn], in_=ps)

        ps_out = psum(1, 2)
        for tt in range(NT):
            n = NTOK[tt]
            ps_t = psum(n, NA)
            nc.tensor.matmul(ps_t, lg[:, tt * 128:tt * 128 + n],
                             ident5[:, :], start=True, stop=True)
            sexp = st_p.tile([128, 1], F32, tag="sexp")
            lse = st_p.tile([128, 1], F32, tag="lse")
            lt = st_p.tile([128, 1], F32, tag="lt")
            suml = st_p.tile([128, 1], F32, tag="suml")
            mx = st_p.tile([128, 1], F32, tag="mx")
            t1 = st_p.tile([128, 1], F32, tag="t1")
            lossv = st_p.tile([128, 1], F32, tag="lossv")
            corr = st_p.tile([128, 1], F32, tag="corr")
            scr5 = st_p.tile([128, NA], F32, tag="scr5")
            scr5b = st_p.tile([128, NA], F32, tag="scr5b")
            nc.scalar.activation(out=scr5[:n, :], in_=ps_t, func=AF.Exp,
                                 accum_out=sexp[:n, :])
            nc.scalar.activation(out=lse[:n, :], in_=sexp[:n, :], func=AF.Ln)
            nc.vector.scalar_tensor_tensor(
                out=scr5b[:n, :], in0=ps_t, scalar=1.0,
                in1=tgt[:n, tt, :], op0=ALU.mult, op1=ALU.mult,
                accum_out=lt[:n, :])
            nc.vector.tensor_reduce(out=suml[:n, :], in_=ps_t,
                                    axis=mybir.AxisListType.X, op=ALU.add)
            nc.vector.tensor_reduce(out=mx[:n, :], in_=ps_t,
                                    axis=mybir.AxisListType.X, op=ALU.max)
            nc.vector.scalar_tensor_tensor(
                out=t1[:n, :], in0=lt[:n, :], scalar=-(1.0 - LS),
                in1=lse[:n, :], op0=ALU.mult, op1=ALU.add)
            nc.vector.scalar_tensor_tensor(
                out=lossv[:n, :], in0=suml[:n, :], scalar=-(LS / NA),
                in1=t1[:n, :], op0=ALU.mult, op1=ALU.add)
            nc.vector.tensor_tensor(out=corr[:n, :], in0=lt[:n, :],
                                    in1=mx[:n, :], op=ALU.is_equal)
            res = st_p.tile([128, 2], F32, tag="res")
            nc.vector.tensor_tensor(out=res[:n, 0:1], in0=lossv[:n, :],
                                    in1=smask[:n, tt:tt + 1], op=ALU.mult)
            nc.vector.tensor_tensor(out=res[:n, 1:2], in0=corr[:n, :],
                                    in1=smask[:n, tt:tt + 1], op=ALU.mult)
            nc.tensor.matmul(ps_out, ones128[:n, :], res[:n, :],
                             start=(tt == 0), stop=(tt == NT - 1))
        osb = st_p.tile([1, 2], F32, tag="osb")
        nc.scalar.copy(out=osb[:], in_=ps_out)
        nc.sync.dma_start(d_out[:, :], osb[:])

    nc.compile()
    return nc


# --------------------------------------------------------------------------
# host-side input preparation
# --------------------------------------------------------------------------

def prep_in_maps(inputs):
    st = np.asarray(inputs["states"])
    ac = np.asarray(inputs["actions"])
    rw = _f32(inputs["rewards"])
    qs = np.asarray(inputs["query_states"])
    ta = np.asarray(inputs["target_actions"])
    wpe = _f32(inputs["wpe"])
    emb_s = _f32(inputs["embed_state"])
    emb_a = _f32(inputs["embed_action"])
    emb_rw = _f32(inputs["embed_reward_w"])
    emb_rb = _f32(inputs["embed_reward_b"])

    sid = st[..., 0] * G + st[..., 1]          # [B, CTX]
    qid = qs[..., 0] * G + qs[..., 1]          # [B]

    # combined embedding table [87, D]
    temb = np.concatenate([emb_s, emb_a, emb_rw.reshape(1, D)], axis=0)

    # wpe_eff with reward bias folded into reward-token rows; padded to LP
    wpe_eff = np.zeros((LP, D), np.float32)
    wpe_eff[:L] = wpe[:L]
    wpe_eff[2:300:3] += emb_rb
    wpe_tok = np.concatenate([wpe_eff, wpe_eff], axis=0)   # [608, D]

    # per-layer folded weights
    layers = {}
    scale = 1.0 / np.sqrt(HD)
    vb_nz = apb_nz = mpb_nz = qkvb_nz = fcb_nz = False
    for i in range(NL):
        g1, b1 = _f32(inputs["ln1_g"][i]), _f32(inputs["ln1_b"][i])
        w_at, b_at = _f32(inputs["attn_w"][i]), _f32(inputs["attn_b"][i])
        wq = g1[:, None] * w_at
        bq = b1 @ w_at + b_at
        wq[:, :D] *= scale
        bq = bq.copy()
        bq[:D] *= scale
        # pair q/k head-pair tiles: [q0,k0,q1,k1,...] so attention head-pair
        # hp depends only on the first 2(hp+1) output tiles of the qkv matmul
        perm = []
        for hp in range(8):
            perm.extend(range(hp * 128, (hp + 1) * 128))
            perm.extend(range(D + hp * 128, D + (hp + 1) * 128))
        wq = np.concatenate([wq[:, perm], wq[:, 2 * D:]], axis=1)
        bq = np.concatenate([bq[perm], bq[2 * D:]])
        g2, b2 = _f32(inputs["ln2_g"][i]), _f32(inputs["ln2_b"][i])
        w_fc, b_fc = _f32(inputs["fc_w"][i]), _f32(inputs["fc_b"][i])
        wf = g2[:, None] * w_fc
        bf = b2 @ w_fc + b_fc
        layers[f"w_qkv_{i}"] = _f8(_pack_pairs(wq * WS, 4, 6))
        bqk = bq[:2 * D]
        qkvb_nz |= bool(np.any(bqk))
        layers[f"b_qkv_{i}"] = _f32(bqk.reshape(16, 128).T)
        bv = bq[2 * D:]
        vb_nz |= bool(np.any(bv))
        layers[f"b_v_{i}"] = _bf16(bv.reshape(1, D))
        wap = _f32(inputs["attn_proj_w"][i])
        bap = _f32(inputs["attn_proj_b"][i])
        apb_nz |= bool(np.any(bap))
        layers[f"w_aproj_{i}"] = _f8(_pack_pairs(wap * WS, 4, 2))
        layers[f"b_aproj_{i}"] = _bf16(bap.reshape(1, D))
        fcb_nz |= bool(np.any(bf))
        layers[f"w_fc_{i}"] = _f8(_pack_pairs(wf * WS, 4, 8))
        layers[f"b_fc_{i}"] = _f32(bf.reshape(32, 128).T)
        wmp = _f32(inputs["mlp_proj_w"][i])
        bmp = _f32(inputs["mlp_proj_b"][i])
        mpb_nz |= bool(np.any(bmp))
        layers[f"w_mproj_{i}"] = _f8(_pack_pairs(wmp * WS, 16, 2))
        layers[f"b_mproj_{i}"] = _bf16(bmp.reshape(1, D))

    gf, bff = _f32(inputs["lnf_g"]), _f32(inputs["lnf_b"])
    wp = gf[:, None] * _f32(inputs["pred_w"])
    bp = bff @ _f32(inputs["pred_w"]) + _f32(inputs["pred_b"])
    predb_nz = bool(np.any(bp))
    w_pred = _bf16(wp.reshape(8, 128, NA).transpose(1, 0, 2))
    b_pred = _bf16(bp.reshape(1, NA))

    flags = (vb_nz, apb_nz, mpb_nz, predb_nz, qkvb_nz, fcb_nz)

    # constants
    ident8 = _f8(np.eye(128, dtype=np.float32))
    ident5 = _f32(np.eye(NA))
    # transposed-scores causal mask: NEG where k (row) > q (col)
    tri = _f32(np.where(np.arange(128)[:, None] > np.arange(128)[None, :],
                        NEG, 0.0))
    # bank-B mask: kc1 diag at cols 0:128, kc2 diag at cols 176:221
    mB = np.zeros((128, 224), np.float32)
    mB[:, 0:128] = tri
    r = np.arange(128)[:, None]
    c = np.arange(45)[None, :]
    mB[:, 176:221] = np.where(r > c, NEG, 0.0)
    maskB = _f32(mB)
    ones1 = _bf16(np.ones((1, 512), np.float32))
    ones128 = _f32(np.ones((128, 1), np.float32))

    # state-position mask [128, NT] over the padded token axis
    pos = np.arange(T) % LP
    smask_tok = ((pos < L) & (pos % 3 == 0)).astype(np.float32)
    smask = np.zeros((128, NT), np.float32)
    for tt in range(NT):
        n = NTOK[tt]
        smask[:n, tt] = smask_tok[tt * 128:tt * 128 + n]

    in_maps = []
    for c_ in range(NCORES):
        bs = [2 * c_, 2 * c_ + 1]
        # one-hot embedding matrix [87, 608] (padded cols stay zero)
        m = np.zeros((EMB, T), np.float32)
        tgt = np.zeros((T, NA), np.float32)
        for s, b in enumerate(bs):
            base = s * LP
            p = np.arange(CTX)
            m[sid[b], base + 3 * p] = 1.0
            m[G * G + ac[b], base + 3 * p + 1] = 1.0
            m[EMB - 1, base + 3 * p + 2] = rw[b]
            m[qid[b], base + 300] = 1.0
            tgt[base + 3 * p, ac[b]] = 1.0
            tgt[base + 300, ta[b]] = 1.0
        im = {
            "m_embT": _bf16(m),
            "t_emb": _bf16(temb),
            "wpe": wpe_tok,
            "w_pred": w_pred,
            "tgt_oh": tgt,
            "smask": smask,
            "ident_f8": ident8,
            "ident5": ident5,
            "tri": tri,
            "maskB": maskB,
            "ones128": ones128,
        }
        for i in range(NL):
            for k in (f"w_qkv_{i}", f"w_aproj_{i}", f"w_fc_{i}",
                      f"w_mproj_{i}"):
                im[k] = layers[k]
            if flags[4]:
                im[f"b_qkv_{i}"] = layers[f"b_qkv_{i}"]
            if flags[5]:
                im[f"b_fc_{i}"] = layers[f"b_fc_{i}"]
            if flags[0]:
                im[f"b_v_{i}"] = layers[f"b_v_{i}"]
            if flags[1]:
                im[f"b_aproj_{i}"] = layers[f"b_aproj_{i}"]
            if flags[2]:
                im[f"b_mproj_{i}"] = layers[f"b_mproj_{i}"]
        if flags[3]:
            im["b_pred"] = b_pred
        if any(flags[:4]):
            im["ones1"] = ones1
        in_maps.append(im)
    return in_maps, flags


_NC_CACHE = {}


def run(inputs, trace=False):
    in_maps, flags = prep_in_maps(inputs)
    if flags not in _NC_CACHE:
        _NC_CACHE[flags] = build(flags)
    nc = _NC_CACHE[flags]
    res = run_bass_kernel_spmd(nc, in_maps, core_ids=list(range(NCORES)),
                               trace=trace)
    tot = np.zeros(2, np.float64)
    for c_ in range(NCORES):
        tot += res.results[c_]["out"].reshape(2).astype(np.float64)
    denom = B * (CTX + 1)
    loss = np.float32(tot[0] / denom)
    acc = np.float32(tot[1] / denom)
    return (loss, acc), res


# --------------------------------------------------------------------------
# harness entry point: full inputs in, full output out
# --------------------------------------------------------------------------

def kernel(**inputs):
    """Decision-transformer forward pass on 8 TRN2 NeuronCores.

    Takes the full (unsharded) inputs of reference.setup_inputs() and
    returns (loss, acc) as float32 scalars, matching reference().
    """
    (loss, acc), _ = run(inputs, trace=False)
    return loss, acc
